# revision 15
# baseline (speedup 1.0000x reference)
"""BinaryConv2d (3x3, stride 1, pad 1) on 8 TRN2 NeuronCores.

Data-parallel: batch 32 sharded 4-per-core; weight/bias replicated.

Algorithm: 1-D Winograd F(2,3) along H. For each pair of output rows
(tile row t) the conv needs 4 H-transformed input rows
  U0 = x[2t-1] - x[2t+1]   U1 = x[2t] + x[2t+1]
  U2 = x[2t+1] - x[2t]     U3 = x[2t] - x[2t+2]
and 4 transformed weight sets Wt[a][o,i,dw] = sum_dh G[a,dh] w[o,i,dh,dw]
(entries +-0.5/+-1.5/+-1, bf16-exact for binarized weights). Then
  V[a] = sum_dw Wt[a][:,:,dw] @ U[a] shifted by dw   (3 matmuls, PSUM)
  y[2t]   = V0 + V1 + V2 + bias
  y[2t+1] = V1 - V2 - V3 + bias
12 matmuls per 14 output rows per 128-channel half instead of the direct
conv's 18 -- 1.5x less PE work, and PE is the bottleneck.

Width padding is avoided entirely: the dw=1 (center) tap runs first at
full width N=392 with start=True, then the dw=0/2 taps accumulate into
column-shifted PSUM windows at N=385 -- the skipped edge column is
exactly the tap's zero-pad contribution.

Engine split (arrived at over ~10 traced iterations):
 - PE: 12 MMs per (group, half) at N=392; weights resident in SBUF, the
   LDWEIGHTS for each hides under the 166ns MM spacing (measured: warm
   steady-state spacing == the 392-cycle streaming floor, zero gaps).
 - DVE: the 4 unavoidable two-tensor output combines per (group, half)
   (max one PSUM operand each -- hw limit) + ALL of image 0's input
   transform pieces (per 7-tile-row piece so MM #1 can start ~10.5us
   in; keeping piece 3 off GpSimd starts image 1's transform ~8us
   earlier, protecting the image-boundary margin when the chip runs in
   its slow/throttled state).
 - GpSimd: input transform for images 1..3 (staged two images ahead so
   its ~3.8us/op rate stays off the critical path) + the weight/bias DMA
   issues (its queue is idle early; DMA issue ops cost ~650ns each and
   would serialize behind the image chunks on Sync).
 - ACT: bias folding (c0 = V0+b, c3 = b-V3, reading PSUM) + xs edge
   memsets.
 - Sync: image DMAs -- fully-contiguous 6272B/partition transfers into
   an H-padded-only slab (a W-padded slab forces 112B-chunk DMA at 1/4
   rate); image 0 is split into 4 row-chunks so the first transform
   piece gates on 229KB only; all output DMAs.
 - PE warmup: 36 dummy matmuls bridge the preamble+fill window so the
   HAM clock-gate (K=4/8 cold throttle) lifts before the real stream.
"""

import numpy as np
from contextlib import ExitStack

import concourse.bass as bass
import concourse.bacc as bacc
import concourse.mybir as mybir
import concourse.tile as tile
from concourse.bass_utils import run_bass_kernel_spmd

N_CORES = 8
N_BATCH = 32
N_PER_CORE = N_BATCH // N_CORES  # 4
C_IN = 128
C_OUT = 256
H = W = 56
HP = H + 2           # zero-padded height (in xs)
T_IMG = H // 2       # 28 tile rows per image
T_GRP = 7            # tile rows per matmul group
NGRP = T_IMG // T_GRP  # 4 groups -> 14 output rows each

f32 = mybir.dt.float32
bf16 = mybir.dt.bfloat16
f16 = mybir.dt.float16
AF = mybir.ActivationFunctionType


def build_program() -> bass.Bass:
    nc = bacc.Bacc("TRN2", target_bir_lowering=False, debug=False)
    x = nc.dram_tensor("x", [N_PER_CORE, C_IN, H, W], bf16, kind="ExternalInput")
    # wt[half, i, a, dw, o]: host-transformed Winograd weights. The values
    # (0, +-0.5, +-1, +-1.5) are exact in fp8-e4m3, which halves the
    # weight-stream bytes (the early-fill gate) at zero accuracy cost;
    # a single-fp8 lhsT against a bf16 rhs runs at bf16 speed.
    fp8 = mybir.dt.float8e4
    wt = nc.dram_tensor("wt", [2, C_IN, 4, 3, 128], fp8, kind="ExternalInput")
    # bias arrives host-packed as [o, half] so its DMA is a contiguous
    # 1KB transfer (the on-chip "(h o) -> o h" gather read 4B-interleaved
    # chunks and landed ~6us after issue, gating the first c0 combine)
    b = nc.dram_tensor("b", [128, 2], f32, kind="ExternalInput")
    y = nc.dram_tensor("y", [N_PER_CORE, C_OUT, H, W], f16, kind="ExternalOutput")

    with tile.TileContext(nc) as tc, ExitStack() as ctx:
        singles = ctx.enter_context(tc.tile_pool(name="singles", bufs=1))
        xsp = ctx.enter_context(tc.tile_pool(name="xsp", bufs=4))
        up = ctx.enter_context(tc.tile_pool(name="up", bufs=3))
        psum_mm = ctx.enter_context(
            tc.tile_pool(name="psum_mm", bufs=8, space="PSUM")
        )
        tdp = ctx.enter_context(tc.tile_pool(name="tdp", bufs=8))
        obp = ctx.enter_context(tc.tile_pool(name="obp", bufs=4))

        # warmup weight tile first: its GpSimd memset is the earliest
        # producer any PE work can gate on
        warm_w = singles.tile([128, 128], bf16)
        nc.gpsimd.memset(warm_w, 0.0)

        wtile = singles.tile([128, 2, 4, 3, 128], mybir.dt.float8e4,
                             name="wt")
        bsb = singles.tile([128, 2], f32)

        def stage_image(n):
            """One contiguous DMA into the H-padded slab + edge memsets +
            the four H-transform ops (GpSimd; staged 2 images ahead)."""
            xs = xsp.tile([128, HP, W], bf16, name="xs")
            # pad-row memzeros BEFORE the DMA: the Tile tracker orders
            # same-tile writes, and memzero-after-DMA makes the ACT queue
            # block on the image DMA completion -- which stalled the PE
            # 1.5us behind the c0/c3 combines queued after it. The
            # reverse order costs nothing (the DMA waits on two ~100ns
            # memzeros done right after allocation).
            nc.scalar.memzero(xs[:, 0, :])
            nc.scalar.memzero(xs[:, HP - 1, :])
            nc.sync.dma_start(out=xs[:, 1:1 + H, :], in_=x.ap()[n])

            def e(k):  # rows k, k+2, ..., k+54 of the padded slab
                return xs[:, k:k + 2 * (T_IMG - 1) + 1:2, :]

            U = [up.tile([128, T_IMG, W], bf16, name=f"u{a}", tag=f"u{a}")
                 for a in range(4)]
            nc.gpsimd.tensor_sub(U[0], e(0), e(2))
            nc.gpsimd.tensor_add(U[1], e(1), e(2))
            nc.gpsimd.tensor_sub(U[2], e(2), e(1))
            nc.gpsimd.tensor_sub(U[3], e(1), e(3))
            return U

        def stage_image0():
            """Image 0 rides the critical path: 4 row-chunk DMAs (first
            matmul gates on a 229KB transfer, not 784KB) + per-group
            U-transform pieces on the DVE; weights/bias issue from the
            GpSimd queue in parallel, (half0, a0) first."""
            xs = xsp.tile([128, HP, W], bf16, name="xs")
            # weights + bias issue from the (idle) GpSimd queue so their
            # ~650ns issue ops don't serialize behind the image chunks
            # on the Sync queue; (half0, a0) goes alone so MM #1's gate
            # is a 196KB transfer
            nc.gpsimd.dma_start(out=wtile[:, 0, 0], in_=wt.ap()[0][:, 0])
            nc.gpsimd.dma_start(out=wtile[:, 0, 1:4], in_=wt.ap()[0][:, 1:4])
            nc.gpsimd.dma_start(out=bsb, in_=b.ap())
            nc.scalar.memzero(xs[:, 0, :])
            nc.scalar.memzero(xs[:, HP - 1, :])
            chunks = [(0, 16), (16, 30), (30, 44), (44, 56)]
            for ci, (lo, hi) in enumerate(chunks):
                nc.sync.dma_start(out=xs[:, 1 + lo:1 + hi, :],
                                  in_=x.ap()[0, :, lo:hi, :])
                if ci == 1:
                    # half1 weights ride the fast HWDGE path: the SWDGE
                    # route's ~2us fixed latency landed them right at (or
                    # after) group0-half1's LDWEIGHTS, a 2.4us PE stall in
                    # the chip's slow-clock state where it isn't hidden by
                    # the HAM ramp
                    nc.sync.dma_start(out=wtile[:, 1], in_=wt.ap()[1])

            U = [up.tile([128, T_IMG, W], bf16, name=f"u{a}", tag=f"u{a}")
                 for a in range(4)]

            def piece(g, eng):
                r = slice(T_GRP * g, T_GRP * (g + 1))

                def e(k):  # rows 14g+k, +2, ..., +12 of the padded slab
                    return xs[:, 14 * g + k:14 * g + k + 13:2, :]

                eng.tensor_sub(U[0][:, r, :], e(0), e(2))
                eng.tensor_add(U[1][:, r, :], e(1), e(2))
                eng.tensor_sub(U[2][:, r, :], e(2), e(1))
                eng.tensor_sub(U[3][:, r, :], e(1), e(3))

            # pieces 0-2 on DVE (fast, ahead of the output-op backlog),
            # the last on GpSimd (idle until image 1's transform; its
            # ~1us/op rate still beats group 3's ~21us deadline)
            for g in range(NGRP):
                piece(g, nc.vector)
            return U

        # ---- PE warmup: bridge the pipeline-fill window (preamble + first
        # image DMA + first U transform) with dummy matmuls so the HAM
        # clock-gate lifts before the real stream starts.
        wp = psum_mm.tile([128, 128], f32, tag="ps")
        NWARM = 32
        for k in range(NWARM):
            nc.tensor.matmul(wp, lhsT=warm_w, rhs=warm_w,
                             start=(k == 0), stop=(k == NWARM - 1))

        def do_group(n, U, g, half, split_dma=False):
            """14 output rows (tile rows 7g..7g+6) of image n, one half."""
            h0 = 2 * T_GRP * g
            r = slice(T_GRP * g, T_GRP * (g + 1))
            V = [None] * 4
            # last group only: a=3 first, so its bias-fold (c3) runs during
            # the remaining matmuls and only y0/y1 trail the last one
            for a in ((3, 0, 1, 2) if split_dma else (0, 1, 2, 3)):
                ps = psum_mm.tile([128, T_GRP, W], f32, name=f"v{a}",
                                  tag="ps")
                lt = wtile[:, half, a]
                # center tap first at full width (sets has_written), then
                # the shifted taps accumulate into partial column windows
                nc.tensor.matmul(ps, lhsT=lt[:, 1], rhs=U[a][:, r, :],
                                 start=True, stop=False)
                nc.tensor.matmul(ps[:, :, 1:W], lhsT=lt[:, 0],
                                 rhs=U[a][:, r, 0:W - 1],
                                 start=False, stop=False)
                nc.tensor.matmul(ps[:, :, 0:W - 1], lhsT=lt[:, 2],
                                 rhs=U[a][:, r, 1:W],
                                 start=False, stop=True)
                V[a] = ps
            # y0 = V0+V1+V2+b, y1 = V1-V2-V3+b; ACT folds the bias into the
            # single-use terms, DVE does the two-tensor combines (max one
            # PSUM operand each).
            ob = obp.tile([128, T_GRP, 2, W], f16, name="ob", tag="ob")
            c0 = tdp.tile([128, T_GRP, W], f32, name="c0", tag="td")
            c3 = tdp.tile([128, T_GRP, W], f32, name="c3", tag="td")
            t = tdp.tile([128, T_GRP, W], f32, name="t", tag="td")
            e = tdp.tile([128, T_GRP, W], f32, name="e", tag="td")

            def act_c0():
                nc.scalar.activation(c0, V[0], AF.Identity,
                                     bias=bsb[:, half:half + 1])

            def act_c3():
                nc.scalar.activation(c3, V[3], AF.Identity,
                                     bias=bsb[:, half:half + 1], scale=-1.0)

            ych = y.ap()[n, half * 128:(half + 1) * 128]
            if split_dma:
                # tail: c3/c0/t/e all complete during the a=0..2 matmuls
                # (a=3 ran first), so only y0/y1 trail the last matmul --
                # and they go in two row-blocks so the first block's DMA
                # overlaps the second block's compute. All transfers stay
                # row-contiguous.
                act_c3(), act_c0()
                nc.vector.tensor_add(t, c0, V[1])
                nc.vector.tensor_add(e, c3, V[1])
                for rlo, rhi in ((0, 4), (4, T_GRP)):
                    nc.vector.tensor_add(ob[:, rlo:rhi, 0, :],
                                         t[:, rlo:rhi], V[2][:, rlo:rhi])
                    nc.vector.tensor_sub(ob[:, rlo:rhi, 1, :],
                                         e[:, rlo:rhi], V[2][:, rlo:rhi])
                    nc.sync.dma_start(
                        out=ych[:, h0 + 2 * rlo:h0 + 2 * rhi, :],
                        in_=ob[:, rlo:rhi],
                    )
            else:
                act_c0()
                nc.vector.tensor_add(t, c0, V[1])
                nc.vector.tensor_add(ob[:, :, 0, :], t, V[2])
                act_c3()
                nc.vector.tensor_add(e, c3, V[1])
                nc.vector.tensor_sub(ob[:, :, 1, :], e, V[2])
                nc.sync.dma_start(out=ych[:, h0:h0 + 2 * T_GRP, :], in_=ob)

        # software pipeline: staged two images ahead so GpSimd's slower
        # transform rate never gates the PE
        Us = [stage_image0(), stage_image(1), stage_image(2)]
        for n in range(N_PER_CORE):
            if n + 3 < N_PER_CORE:
                Us.append(stage_image(n + 3))
            for g in range(NGRP):
                for half in range(2):
                    last = (n == N_PER_CORE - 1 and g == NGRP - 1
                            and half == 1)
                    do_group(n, Us[n], g, half, split_dma=last)
    nc.compile()
    return nc


# F(2,3) weight transform G (exact in bf16 for +-1 weights)
_G = np.array([[1, 0, 0], [0.5, 0.5, 0.5], [0.5, -0.5, 0.5], [0, 0, 1]],
              dtype=np.float32)


def host_weight_layout(weight: np.ndarray) -> np.ndarray:
    """[256, 128, 3, 3] -> binarize, G-transform along dh,
    layout [half, i, a, dw, o] = [2, 128, 4, 3, 128] fp8-e4m3
    (values 0/+-0.5/+-1/+-1.5 are e4m3-exact)."""
    import ml_dtypes
    wc = np.clip(weight.astype(np.float32), -1.0, 1.0)
    wbin = np.where(wc >= 0, 1.0, -1.0).astype(np.float32)
    wtr = np.einsum("ad,oidw->aoiw", _G, wbin)     # [a, o, i, dw]
    w5 = wtr.reshape(4, 2, 128, C_IN, 3)           # [a, half, oo, i, dw]
    w6 = w5.transpose(1, 3, 0, 4, 2)               # [half, i, a, dw, oo]
    return np.ascontiguousarray(w6).astype(ml_dtypes.float8_e4m3fn)


def run(x, weight, bias, trace=False):
    """Returns (out [32,256,56,56] f32, BassKernelResults)."""
    import ml_dtypes
    nc = build_program()
    xb = np.asarray(x, dtype=np.float32).astype(ml_dtypes.bfloat16)
    wtr = host_weight_layout(np.asarray(weight))
    bias = np.ascontiguousarray(
        np.asarray(bias, dtype=np.float32).reshape(2, 128).T)
    in_maps = [
        {
            "x": xb[i * N_PER_CORE:(i + 1) * N_PER_CORE],
            "wt": wtr,
            "b": bias,
        }
        for i in range(N_CORES)
    ]
    res = run_bass_kernel_spmd(
        nc, in_maps, core_ids=list(range(N_CORES)), trace=trace
    )
    out = np.concatenate([r["y"] for r in res.results], axis=0)
    return out.astype(np.float32), res


def kernel(x, weight, bias):
    out, _ = run(x, weight, bias)
    return out



# revision 16
# speedup vs baseline: 1.0238x; 1.0238x over previous
"""BinaryConv2d (3x3, stride 1, pad 1) on 8 TRN2 NeuronCores.

Data-parallel: batch 32 sharded 4-per-core; weight/bias replicated.

Algorithm: 1-D Winograd F(2,3) along H. For each pair of output rows
(tile row t) the conv needs 4 H-transformed input rows
  U0 = x[2t-1] - x[2t+1]   U1 = x[2t] + x[2t+1]
  U2 = x[2t+1] - x[2t]     U3 = x[2t] - x[2t+2]
and 4 transformed weight sets Wt[a][o,i,dw] = sum_dh G[a,dh] w[o,i,dh,dw]
(entries +-0.5/+-1.5/+-1, bf16-exact for binarized weights). Then
  V[a] = sum_dw Wt[a][:,:,dw] @ U[a] shifted by dw   (3 matmuls, PSUM)
  y[2t]   = V0 + V1 + V2 + bias
  y[2t+1] = V1 - V2 - V3 + bias
12 matmuls per 14 output rows per 128-channel half instead of the direct
conv's 18 -- 1.5x less PE work, and PE is the bottleneck.

Width padding is avoided entirely: the dw=1 (center) tap runs first at
full width N=392 with start=True, then the dw=0/2 taps accumulate into
column-shifted PSUM windows at N=385 -- the skipped edge column is
exactly the tap's zero-pad contribution.

Engine split (arrived at over ~10 traced iterations):
 - PE: 12 MMs per (group, half) at N=392; weights resident in SBUF, the
   LDWEIGHTS for each hides under the 166ns MM spacing (measured: warm
   steady-state spacing == the 392-cycle streaming floor, zero gaps).
 - DVE: the 4 unavoidable two-tensor output combines per (group, half)
   (max one PSUM operand each -- hw limit) + ALL of image 0's input
   transform pieces (per 7-tile-row piece so MM #1 can start ~10.5us
   in; keeping piece 3 off GpSimd starts image 1's transform ~8us
   earlier, protecting the image-boundary margin when the chip runs in
   its slow/throttled state).
 - GpSimd: input transform for images 1..3 (staged two images ahead so
   its ~3.8us/op rate stays off the critical path) + the weight/bias DMA
   issues (its queue is idle early; DMA issue ops cost ~650ns each and
   would serialize behind the image chunks on Sync).
 - ACT: bias folding (c0 = V0+b, c3 = b-V3, reading PSUM) + xs edge
   memsets.
 - Sync: image DMAs -- fully-contiguous 6272B/partition transfers into
   an H-padded-only slab (a W-padded slab forces 112B-chunk DMA at 1/4
   rate); image 0 is split into 4 row-chunks so the first transform
   piece gates on 229KB only; all output DMAs.
 - PE warmup: 36 dummy matmuls bridge the preamble+fill window so the
   HAM clock-gate (K=4/8 cold throttle) lifts before the real stream.
"""

import numpy as np
from contextlib import ExitStack

import concourse.bass as bass
import concourse.bacc as bacc
import concourse.mybir as mybir
import concourse.tile as tile
from concourse.bass_utils import run_bass_kernel_spmd

N_CORES = 8
N_BATCH = 32
N_PER_CORE = N_BATCH // N_CORES  # 4
C_IN = 128
C_OUT = 256
H = W = 56
HP = H + 2           # zero-padded height (in xs)
T_IMG = H // 2       # 28 tile rows per image
T_GRP = 7            # tile rows per matmul group
NGRP = T_IMG // T_GRP  # 4 groups -> 14 output rows each

f32 = mybir.dt.float32
bf16 = mybir.dt.bfloat16
f16 = mybir.dt.float16
AF = mybir.ActivationFunctionType


def build_program() -> bass.Bass:
    nc = bacc.Bacc("TRN2", target_bir_lowering=False, debug=False)
    x = nc.dram_tensor("x", [N_PER_CORE, C_IN, H, W], bf16, kind="ExternalInput")
    # wt[half, i, a, dw, o]: host-transformed Winograd weights. The values
    # (0, +-0.5, +-1, +-1.5) are exact in fp8-e4m3, which halves the
    # weight-stream bytes (the early-fill gate) at zero accuracy cost;
    # a single-fp8 lhsT against a bf16 rhs runs at bf16 speed.
    fp8 = mybir.dt.float8e4
    wt = nc.dram_tensor("wt", [2, C_IN, 4, 3, 128], fp8, kind="ExternalInput")
    # bias arrives host-packed as [o, half] so its DMA is a contiguous
    # 1KB transfer (the on-chip "(h o) -> o h" gather read 4B-interleaved
    # chunks and landed ~6us after issue, gating the first c0 combine)
    b = nc.dram_tensor("b", [128, 2], f32, kind="ExternalInput")
    y = nc.dram_tensor("y", [N_PER_CORE, C_OUT, H, W], f16, kind="ExternalOutput")

    with tile.TileContext(nc) as tc, ExitStack() as ctx:
        singles = ctx.enter_context(tc.tile_pool(name="singles", bufs=1))
        xsp = ctx.enter_context(tc.tile_pool(name="xsp", bufs=4))
        up = ctx.enter_context(tc.tile_pool(name="up", bufs=3))
        psum_mm = ctx.enter_context(
            tc.tile_pool(name="psum_mm", bufs=8, space="PSUM")
        )
        tdp = ctx.enter_context(tc.tile_pool(name="tdp", bufs=8))
        obp = ctx.enter_context(tc.tile_pool(name="obp", bufs=4))

        # warmup weight tile first: its GpSimd memset is the earliest
        # producer any PE work can gate on
        warm_w = singles.tile([128, 128], bf16)
        nc.gpsimd.memset(warm_w, 0.0)

        wtile = singles.tile([128, 2, 4, 3, 128], mybir.dt.float8e4,
                             name="wt")
        bsb = singles.tile([128, 2], f32)

        def stage_image(n):
            """One contiguous DMA into the H-padded slab + edge memsets +
            the four H-transform ops (GpSimd; staged 2 images ahead)."""
            xs = xsp.tile([128, HP, W], bf16, name="xs")
            # pad-row memzeros BEFORE the DMA: the Tile tracker orders
            # same-tile writes, and memzero-after-DMA makes the ACT queue
            # block on the image DMA completion -- which stalled the PE
            # 1.5us behind the c0/c3 combines queued after it. The
            # reverse order costs nothing (the DMA waits on two ~100ns
            # memzeros done right after allocation).
            nc.scalar.memzero(xs[:, 0, :])
            nc.scalar.memzero(xs[:, HP - 1, :])
            nc.sync.dma_start(out=xs[:, 1:1 + H, :], in_=x.ap()[n])

            def e(k):  # rows k, k+2, ..., k+54 of the padded slab
                return xs[:, k:k + 2 * (T_IMG - 1) + 1:2, :]

            U = [up.tile([128, T_IMG, W], bf16, name=f"u{a}", tag=f"u{a}")
                 for a in range(4)]
            nc.gpsimd.tensor_sub(U[0], e(0), e(2))
            nc.gpsimd.tensor_add(U[1], e(1), e(2))
            nc.gpsimd.tensor_sub(U[2], e(2), e(1))
            nc.gpsimd.tensor_sub(U[3], e(1), e(3))
            return U

        def stage_image_split(n):
            """Like stage_image but image n arrives in two row-chunks and
            transforms in two half-slab pieces, so its first U ops finish
            ~5us earlier -- image 1's transform otherwise gates the
            image-boundary matmuls (MM#137-class ~1.7us stall)."""
            xs = xsp.tile([128, HP, W], bf16, name="xs")
            nc.scalar.memzero(xs[:, 0, :])
            nc.scalar.memzero(xs[:, HP - 1, :])
            nc.sync.dma_start(out=xs[:, 1:29, :], in_=x.ap()[n, :, 0:28, :])
            nc.sync.dma_start(out=xs[:, 29:57, :], in_=x.ap()[n, :, 28:56, :])
            U = [up.tile([128, T_IMG, W], bf16, name=f"u{a}", tag=f"u{a}")
                 for a in range(4)]
            for t0, t1 in ((0, 14), (14, T_IMG)):
                r = slice(t0, t1)

                def e(k):
                    lo = 2 * t0 + k
                    return xs[:, lo:lo + 2 * (t1 - t0 - 1) + 1:2, :]

                nc.gpsimd.tensor_sub(U[0][:, r, :], e(0), e(2))
                nc.gpsimd.tensor_add(U[1][:, r, :], e(1), e(2))
                nc.gpsimd.tensor_sub(U[2][:, r, :], e(2), e(1))
                nc.gpsimd.tensor_sub(U[3][:, r, :], e(1), e(3))
            return U

        def stage_image0():
            """Image 0 rides the critical path: 4 row-chunk DMAs (first
            matmul gates on a 229KB transfer, not 784KB) + per-group
            U-transform pieces on the DVE; weights/bias issue from the
            GpSimd queue in parallel, (half0, a0) first."""
            xs = xsp.tile([128, HP, W], bf16, name="xs")
            # weights + bias issue from the (idle) GpSimd queue so their
            # ~650ns issue ops don't serialize behind the image chunks
            # on the Sync queue; (half0, a0) goes alone so MM #1's gate
            # is a 196KB transfer
            nc.gpsimd.dma_start(out=wtile[:, 0, 0], in_=wt.ap()[0][:, 0])
            nc.gpsimd.dma_start(out=wtile[:, 0, 1:4], in_=wt.ap()[0][:, 1:4])
            nc.gpsimd.dma_start(out=bsb, in_=b.ap())
            nc.scalar.memzero(xs[:, 0, :])
            nc.scalar.memzero(xs[:, HP - 1, :])
            chunks = [(0, 16), (16, 30), (30, 44), (44, 56)]
            for ci, (lo, hi) in enumerate(chunks):
                nc.sync.dma_start(out=xs[:, 1 + lo:1 + hi, :],
                                  in_=x.ap()[0, :, lo:hi, :])
                if ci == 1:
                    # half1 weights ride the fast HWDGE path: the SWDGE
                    # route's ~2us fixed latency landed them right at (or
                    # after) group0-half1's LDWEIGHTS, a 2.4us PE stall in
                    # the chip's slow-clock state where it isn't hidden by
                    # the HAM ramp
                    nc.sync.dma_start(out=wtile[:, 1], in_=wt.ap()[1])

            U = [up.tile([128, T_IMG, W], bf16, name=f"u{a}", tag=f"u{a}")
                 for a in range(4)]

            def piece(g, eng):
                r = slice(T_GRP * g, T_GRP * (g + 1))

                def e(k):  # rows 14g+k, +2, ..., +12 of the padded slab
                    return xs[:, 14 * g + k:14 * g + k + 13:2, :]

                eng.tensor_sub(U[0][:, r, :], e(0), e(2))
                eng.tensor_add(U[1][:, r, :], e(1), e(2))
                eng.tensor_sub(U[2][:, r, :], e(2), e(1))
                eng.tensor_sub(U[3][:, r, :], e(1), e(3))

            # pieces 0-2 on DVE (fast, ahead of the output-op backlog),
            # the last on GpSimd (idle until image 1's transform; its
            # ~1us/op rate still beats group 3's ~21us deadline)
            for g in range(NGRP):
                piece(g, nc.vector)
            return U

        # ---- PE warmup: bridge the pipeline-fill window (preamble + first
        # image DMA + first U transform) with dummy matmuls so the HAM
        # clock-gate lifts before the real stream starts.
        wp = psum_mm.tile([128, 128], f32, tag="ps")
        NWARM = 32
        for k in range(NWARM):
            nc.tensor.matmul(wp, lhsT=warm_w, rhs=warm_w,
                             start=(k == 0), stop=(k == NWARM - 1))

        def do_group(n, U, g, half, split_dma=False):
            """14 output rows (tile rows 7g..7g+6) of image n, one half."""
            h0 = 2 * T_GRP * g
            r = slice(T_GRP * g, T_GRP * (g + 1))
            V = [None] * 4
            # last group only: a=3 first, so its bias-fold (c3) runs during
            # the remaining matmuls and only y0/y1 trail the last one
            for a in ((3, 0, 1, 2) if split_dma else (0, 1, 2, 3)):
                ps = psum_mm.tile([128, T_GRP, W], f32, name=f"v{a}",
                                  tag="ps")
                lt = wtile[:, half, a]
                # center tap first at full width (sets has_written), then
                # the shifted taps accumulate into partial column windows
                nc.tensor.matmul(ps, lhsT=lt[:, 1], rhs=U[a][:, r, :],
                                 start=True, stop=False)
                nc.tensor.matmul(ps[:, :, 1:W], lhsT=lt[:, 0],
                                 rhs=U[a][:, r, 0:W - 1],
                                 start=False, stop=False)
                nc.tensor.matmul(ps[:, :, 0:W - 1], lhsT=lt[:, 2],
                                 rhs=U[a][:, r, 1:W],
                                 start=False, stop=True)
                V[a] = ps
            # y0 = V0+V1+V2+b, y1 = V1-V2-V3+b; ACT folds the bias into the
            # single-use terms, DVE does the two-tensor combines (max one
            # PSUM operand each).
            ob = obp.tile([128, T_GRP, 2, W], f16, name="ob", tag="ob")
            c0 = tdp.tile([128, T_GRP, W], f32, name="c0", tag="td")
            c3 = tdp.tile([128, T_GRP, W], f32, name="c3", tag="td")
            t = tdp.tile([128, T_GRP, W], f32, name="t", tag="td")
            e = tdp.tile([128, T_GRP, W], f32, name="e", tag="td")

            def act_c0():
                nc.scalar.activation(c0, V[0], AF.Identity,
                                     bias=bsb[:, half:half + 1])

            def act_c3():
                nc.scalar.activation(c3, V[3], AF.Identity,
                                     bias=bsb[:, half:half + 1], scale=-1.0)

            ych = y.ap()[n, half * 128:(half + 1) * 128]
            if split_dma:
                # tail: c3/c0/t/e all complete during the a=0..2 matmuls
                # (a=3 ran first), so only y0/y1 trail the last matmul --
                # and they go in two row-blocks so the first block's DMA
                # overlaps the second block's compute. All transfers stay
                # row-contiguous.
                act_c3(), act_c0()
                nc.vector.tensor_add(t, c0, V[1])
                nc.vector.tensor_add(e, c3, V[1])
                for rlo, rhi in ((0, 4), (4, T_GRP)):
                    nc.vector.tensor_add(ob[:, rlo:rhi, 0, :],
                                         t[:, rlo:rhi], V[2][:, rlo:rhi])
                    nc.vector.tensor_sub(ob[:, rlo:rhi, 1, :],
                                         e[:, rlo:rhi], V[2][:, rlo:rhi])
                    nc.sync.dma_start(
                        out=ych[:, h0 + 2 * rlo:h0 + 2 * rhi, :],
                        in_=ob[:, rlo:rhi],
                    )
            else:
                act_c0()
                nc.vector.tensor_add(t, c0, V[1])
                nc.vector.tensor_add(ob[:, :, 0, :], t, V[2])
                act_c3()
                nc.vector.tensor_add(e, c3, V[1])
                nc.vector.tensor_sub(ob[:, :, 1, :], e, V[2])
                nc.sync.dma_start(out=ych[:, h0:h0 + 2 * T_GRP, :], in_=ob)

        # software pipeline: staged two images ahead so GpSimd's slower
        # transform rate never gates the PE
        Us = [stage_image0(), stage_image_split(1), stage_image(2)]
        for n in range(N_PER_CORE):
            if n + 3 < N_PER_CORE:
                Us.append(stage_image(n + 3))
            for g in range(NGRP):
                for half in range(2):
                    last = (n == N_PER_CORE - 1 and g == NGRP - 1
                            and half == 1)
                    do_group(n, Us[n], g, half, split_dma=last)
    nc.compile()
    return nc


# F(2,3) weight transform G (exact in bf16 for +-1 weights)
_G = np.array([[1, 0, 0], [0.5, 0.5, 0.5], [0.5, -0.5, 0.5], [0, 0, 1]],
              dtype=np.float32)


def host_weight_layout(weight: np.ndarray) -> np.ndarray:
    """[256, 128, 3, 3] -> binarize, G-transform along dh,
    layout [half, i, a, dw, o] = [2, 128, 4, 3, 128] fp8-e4m3
    (values 0/+-0.5/+-1/+-1.5 are e4m3-exact)."""
    import ml_dtypes
    wc = np.clip(weight.astype(np.float32), -1.0, 1.0)
    wbin = np.where(wc >= 0, 1.0, -1.0).astype(np.float32)
    wtr = np.einsum("ad,oidw->aoiw", _G, wbin)     # [a, o, i, dw]
    w5 = wtr.reshape(4, 2, 128, C_IN, 3)           # [a, half, oo, i, dw]
    w6 = w5.transpose(1, 3, 0, 4, 2)               # [half, i, a, dw, oo]
    return np.ascontiguousarray(w6).astype(ml_dtypes.float8_e4m3fn)


def run(x, weight, bias, trace=False):
    """Returns (out [32,256,56,56] f32, BassKernelResults)."""
    import ml_dtypes
    nc = build_program()
    xb = np.asarray(x, dtype=np.float32).astype(ml_dtypes.bfloat16)
    wtr = host_weight_layout(np.asarray(weight))
    bias = np.ascontiguousarray(
        np.asarray(bias, dtype=np.float32).reshape(2, 128).T)
    in_maps = [
        {
            "x": xb[i * N_PER_CORE:(i + 1) * N_PER_CORE],
            "wt": wtr,
            "b": bias,
        }
        for i in range(N_CORES)
    ]
    res = run_bass_kernel_spmd(
        nc, in_maps, core_ids=list(range(N_CORES)), trace=trace
    )
    out = np.concatenate([r["y"] for r in res.results], axis=0)
    return out.astype(np.float32), res


def kernel(x, weight, bias):
    out, _ = run(x, weight, bias)
    return out

